# revision 11
# baseline (speedup 1.0000x reference)
"""BitLinear TRN2 kernel: out = layernorm(x) @ sign(w).T + bias.

Tensor-parallel over out_features, 8 cores. Transfer-lean contract:
each core ships only its 1/8 token shard of x^T (fp16, plus an fp8
copy of the low-d half in 'dr' mode) and its sign weights BIT-PACKED
(1 bit/weight); the full x^T is assembled ON DEVICE by chunked
AllGathers over NeuronLink (8 chunks, pipelined so matmuls start after
the first), and sign weights are unpacked on-device by DVE shift/and +
affine into resident 16-bit/fp8 operands (hidden under the first
gather). Output returns fp16; host casts to f32. Per-call axon traffic
~0.58GB vs ~2.2GB for naive full replication.

LN is folded around the matmul on raw x:
  out[t,o] = (x@bw.T)[t,o] * inv_t + a_t * S[o] + bias[o]
with S[o] = sum_d bw[o,d], inv_t = 1/(std_t+eps), a_t = -mu_t*inv_t;
stats/prep ride the host, inv/a arrive arranged [128, 64] (partition =
token%128, col = token chunk). a_t*S[o]+bias[o] is built once per
token chunk (cb); each PSUM eviction is psum*inv (+cb) straight to
fp16.

Modes (BITLIN_MODE):
  f16 — all-fp16 matmuls: stationary = gathered-x^T k-tile [128, 128],
        moving = weight slice [128, 512] (one fp32 PSUM bank), 32-deep
        k accumulation. 8192 MMs/core ~= the 16-bit PE roofline
        (~1.75 ms). rel err ~6e-4.
  dr  — (default) d < 2048 runs as single-fp8 DoubleRow pairs (2
        contraction rows/cycle, 8 MMs of K=256 at half cost), the rest
        fp16: MM stream ~0.63x of f16. rel err ~1.5e-2 (measured on
        the fixed seed-0 inputs) vs the 2e-2 gate.
"""

import os

import numpy as np
import ml_dtypes

import concourse.bass as bass
import concourse.tile as tile
from concourse import bacc
from concourse import mybir
from concourse.bass_utils import run_bass_kernel_spmd

F32 = mybir.dt.float32
F16 = mybir.dt.float16
FP8 = mybir.dt.float8e4
U8 = mybir.dt.uint8

NP_FP8 = ml_dtypes.float8_e4m3

T, D, O_FULL, NCORES = 8192, 4096, 16384, 8
O = O_FULL // NCORES  # 2048 out-features per core
TS = T // NCORES  # 1024 tokens shipped per core
OB = O // 8  # 256 packed-weight bytes per row
EPS = 1e-5

KT = D // 128  # 32 k-tiles
MC = T // 128  # 64 token chunks
NG = 8  # x^T AllGather chunks (pipelined with compute)
GT = TS // NG  # 128 local tokens per gather chunk == superblock == chunk
WN = 512  # moving free width (one PSUM bank at fp32)
NWCH = O // WN  # 4 out chunks

DF = 2048  # 'dr' mode: d < DF contracts as single-fp8 DoubleRow pairs
KQ = DF // 256  # 8 DoubleRow MMs (K=256 virtual each)
KH = (D - DF) // 128  # 16 fp16 k-tiles over d >= DF

MODE = os.environ.get("BITLIN_MODE", "dr")


def _build(mode):
    dr = mode == "dr"
    nc = bacc.Bacc("TRN2", target_bir_lowering=False, debug=False)
    # fp16 x^T rows: all of d in f16 mode; d >= DF in dr mode
    DH = D - DF if dr else D
    xt_d = nc.declare_dram_parameter("xt", [DH, TS], F16, isOutput=False)
    if dr:
        xq_d = nc.declare_dram_parameter("xq", [DF, TS], FP8, isOutput=False)
    w1_d = nc.declare_dram_parameter("w1", [D, OB], U8, isOutput=False)
    s_d = nc.declare_dram_parameter("srow", [O], F16, isOutput=False)
    b_d = nc.declare_dram_parameter("brow", [O], F16, isOutput=False)
    iv_d = nc.declare_dram_parameter("iv", [128, MC], F32, isOutput=False)
    aa_d = nc.declare_dram_parameter("aa", [128, MC], F32, isOutput=False)
    out_d = nc.declare_dram_parameter("out", [T, O], F16, isOutput=True)

    with tile.TileContext(nc) as tc:
        with (
            tc.tile_pool(name="dram", bufs=1, space="DRAM") as dram,
            tc.tile_pool(name="singles", bufs=1) as singles,
            tc.tile_pool(name="w1p", bufs=2) as w1p,
            tc.tile_pool(name="xtp", bufs=2) as xtp,
            tc.tile_pool(name="cbp", bufs=2) as cbp,
            tc.tile_pool(name="evp", bufs=4) as evp,
            tc.tile_pool(name="mmp", bufs=3, space="PSUM") as mmp,
        ):
            # resident sign(w)^T: fp16 half (+ fp8 DoubleRow half in dr mode)
            KTH = KH if dr else KT
            wt = singles.tile([128, KTH, O], F16)
            if dr:
                # dim1 = kk*2 + h; row r = kk*256 + h*128 + p <-> d = r
                wq = singles.tile([128, 2 * KQ, O], FP8)
            s_bc = singles.tile([128, O], F16)  # S broadcast across partitions
            b_bc = singles.tile([128, O], F16)  # bias broadcast
            iv_sb = singles.tile([128, MC], F32)
            aa_sb = singles.tile([128, MC], F32)

            # ---- x^T shard -> bounce -> chunked AllGather over the 8 cores
            gxts, gxqs = [], []
            for j in range(NG):
                if dr:
                    ibq = dram.tile([DF, GT], FP8, tag=f"ibq{j}", name=f"ibq{j}")
                    gxq = dram.tile(
                        [NCORES * DF, GT],
                        FP8,
                        tag=f"gxq{j}",
                        name=f"gxq{j}",
                        addr_space="Shared",
                    )
                    nc.gpsimd.dma_start(
                        out=ibq[:], in_=xq_d[:, j * GT : (j + 1) * GT]
                    )
                    nc.gpsimd.collective_compute(
                        "AllGather",
                        mybir.AluOpType.bypass,
                        replica_groups=[list(range(NCORES))],
                        ins=[ibq.opt()],
                        outs=[gxq.opt()],
                    )
                    gxqs.append(gxq)
                ibt = dram.tile([DH, GT], F16, tag=f"ibt{j}", name=f"ibt{j}")
                gxt = dram.tile(
                    [NCORES * DH, GT],
                    F16,
                    tag=f"gxt{j}",
                    name=f"gxt{j}",
                    addr_space="Shared",
                )
                nc.gpsimd.dma_start(out=ibt[:], in_=xt_d[:, j * GT : (j + 1) * GT])
                nc.gpsimd.collective_compute(
                    "AllGather",
                    mybir.AluOpType.bypass,
                    replica_groups=[list(range(NCORES))],
                    ins=[ibt.opt()],
                    outs=[gxt.opt()],
                )
                gxts.append(gxt)

            # ---- prep: tiny stat/bias loads + partition-broadcast rows ----
            nc.gpsimd.dma_start(out=iv_sb[:], in_=iv_d[:, :])
            nc.gpsimd.dma_start(out=aa_sb[:], in_=aa_d[:, :])
            sap = s_d[:]
            nc.gpsimd.dma_start(
                out=s_bc[:],
                in_=bass.AP(tensor=sap.tensor, offset=sap.offset, ap=[[0, 128]] + sap.ap),
            )
            bap = b_d[:]
            nc.gpsimd.dma_start(
                out=b_bc[:],
                in_=bass.AP(tensor=bap.tensor, offset=bap.offset, ap=[[0, 128]] + bap.ap),
            )

            # ---- prep: land packed sign bits, unpack to resident operands
            # byte j bit b of row d <-> sign(w)[o = b*256 + j, d] < 0
            def unpack_rows(dst, r0, nrt, to_dtype):
                # dst[:, r, b*OB:(b+1)*OB] <- +-1 of w1 rows [(r0+r)*128, +128)
                for r in range(nrt):
                    w1 = w1p.tile([128, OB], U8, tag="w1")
                    nc.gpsimd.dma_start(
                        out=w1[:],
                        in_=w1_d[(r0 + r) * 128 : (r0 + r + 1) * 128, :],
                    )
                    for b in range(8):
                        tb = w1p.tile([128, OB], U8, tag="tb")
                        nc.vector.tensor_scalar(
                            tb[:],
                            w1[:],
                            b,
                            1,
                            op0=mybir.AluOpType.logical_shift_right,
                            op1=mybir.AluOpType.bitwise_and,
                        )
                        nc.vector.tensor_scalar(
                            dst[:, r, b * OB : (b + 1) * OB],
                            tb[:],
                            -2.0,
                            1.0,
                            op0=mybir.AluOpType.mult,
                            op1=mybir.AluOpType.add,
                        )

            if dr:
                unpack_rows(wq, 0, 2 * KQ, FP8)  # rows d < DF -> fp8 pairs
                unpack_rows(wt, DF // 128, KH, F16)  # rows d >= DF -> fp16
            else:
                unpack_rows(wt, 0, KT, F16)

            # ---- main: stream gathered x^T superblocks, matmul, fused evict
            for j in range(NG):
                for cg in range(NCORES):
                    m = cg * NG + j  # token chunk (tokens cg*TS + j*GT ..)
                    xtb = xtp.tile([128, KTH, GT], F16, tag="xtb")
                    gap = gxts[j][:]
                    nc.gpsimd.dma_start(
                        out=xtb[:],
                        in_=bass.AP(
                            tensor=gap.tensor,
                            offset=gap.offset + cg * DH * GT,
                            ap=[[GT, 128], [128 * GT, KTH], [1, GT]],
                        ),
                    )
                    if dr:
                        xqb = xtp.tile([128, 2 * KQ, GT], FP8, tag="xqb")
                        qap = gxqs[j][:]
                        nc.gpsimd.dma_start(
                            out=xqb[:],
                            in_=bass.AP(
                                tensor=qap.tensor,
                                offset=qap.offset + cg * DF * GT,
                                ap=[[GT, 128], [128 * GT, 2 * KQ], [1, GT]],
                            ),
                        )
                    # cb[p, o] = a_t * S[o] + bias[o] for this token chunk
                    cb = cbp.tile([128, O], F16, tag="cb")
                    nc.vector.tensor_scalar_mul(cb[:], s_bc[:], aa_sb[:, m : m + 1])
                    nc.vector.tensor_add(cb[:], cb[:], b_bc[:])
                    for oc in range(NWCH):
                        osl = slice(oc * WN, (oc + 1) * WN)
                        pm = mmp.tile([128, WN], F32, tag="mm")
                        if dr:
                            for kk in range(KQ):
                                nc.tensor.matmul(
                                    pm[:],
                                    xqb[:, 2 * kk : 2 * kk + 2, :],
                                    wq[:, 2 * kk : 2 * kk + 2, osl],
                                    start=(kk == 0),
                                    stop=False,
                                    perf_mode=mybir.MatmulPerfMode.DoubleRow,
                                    skip_group_check=True,
                                )
                        for k in range(KTH):
                            nc.tensor.matmul(
                                pm[:],
                                xtb[:, k, :],
                                wt[:, k, osl],
                                start=(k == 0 and not dr),
                                stop=(k == KTH - 1),
                                skip_group_check=True,
                            )
                        ev = evp.tile([128, WN], F16, tag="ev")
                        nc.vector.tensor_scalar_mul(ev[:], pm[:], iv_sb[:, m : m + 1])
                        nc.vector.tensor_add(ev[:], ev[:], cb[:, osl])
                        nc.gpsimd.dma_start(
                            out=out_d[m * 128 : (m + 1) * 128, osl], in_=ev[:]
                        )
    nc.compile()
    return nc


_NC_CACHE = {}
LAST_RESULTS = None


def kernel(x, weight, bias):
    global LAST_RESULTS
    import time as _time

    _tv = os.environ.get("BITLIN_TIME", "0") == "1"
    _t0 = _time.time()
    x = np.asarray(x, dtype=np.float32)
    weight = np.asarray(weight, dtype=np.float32)
    bias = np.asarray(bias, dtype=np.float32)
    dr = MODE == "dr"

    # LN stats folded to a per-token affine: out = (x@bw.T)*inv + a*S + b
    mu = x.mean(axis=1, dtype=np.float64)
    sd = np.sqrt(x.var(axis=1, ddof=1, dtype=np.float64))
    inv = (1.0 / (sd + EPS)).astype(np.float32)
    aa = (-mu * inv).astype(np.float32)
    iv_t = np.ascontiguousarray(inv.reshape(MC, 128).T)  # [128, chunk]
    aa_t = np.ascontiguousarray(aa.reshape(MC, 128).T)

    if dr:
        xT = x[:, DF:].T.astype(np.float16, order="C")  # [D-DF, T]
        xQ = x[:, :DF].T.astype(NP_FP8, order="C")  # [DF, T] single fp8
    else:
        xT = x.T.astype(np.float16, order="C")  # [D, T] fp16

    neg = weight < 0  # sign bit; reference asserts no exact zeros
    S = (D - 2 * neg.sum(axis=1, dtype=np.int32)).astype(np.float16)  # [O_FULL]
    b16 = bias.astype(np.float16)
    # pack: byte j bit b of (core i, row d) <-> neg[i*2048 + b*256 + j, d]
    pk = np.packbits(neg.reshape(NCORES, 8, OB, D), axis=1, bitorder="little")
    pk = pk[:, 0]  # [NCORES, OB, D]

    if _tv:
        print(f"[bitlin] host prep: {_time.time() - _t0:.2f}s", flush=True)
    if MODE not in _NC_CACHE:
        _NC_CACHE[MODE] = _build(MODE)
    nc = _NC_CACHE[MODE]
    _t1 = _time.time()

    in_maps = []
    for i in range(NCORES):
        m = {
            "xt": xT[:, i * TS : (i + 1) * TS],
            "w1": pk[i].T,
            "srow": S[i * O : (i + 1) * O],
            "brow": b16[i * O : (i + 1) * O],
            "iv": iv_t,
            "aa": aa_t,
        }
        if dr:
            m["xq"] = xQ[:, i * TS : (i + 1) * TS]
        in_maps.append(m)
    res = run_bass_kernel_spmd(nc, in_maps, list(range(NCORES)))
    LAST_RESULTS = res
    if _tv:
        print(f"[bitlin] run_bass: {_time.time() - _t1:.2f}s", flush=True)
    out = np.empty((T, O_FULL), np.float32)
    for i in range(NCORES):
        out[:, i * O : (i + 1) * O] = res.results[i]["out"]
    if _tv:
        print(f"[bitlin] out assemble: {_time.time() - _t1:.2f}s cum", flush=True)
    return out


# revision 14
# speedup vs baseline: 1.2498x; 1.2498x over previous
"""BitLinear TRN2 kernel: out = layernorm(x) @ sign(w).T + bias.

Tensor-parallel over out_features, 8 cores. Transfer-lean contract:
each core ships only its 1/8 token shard of x^T (fp16, plus an fp8
copy of the low-d half in 'dr' mode) and its sign weights BIT-PACKED
(1 bit/weight); the full x^T is assembled ON DEVICE by chunked
AllGathers over NeuronLink (8 chunks, pipelined so matmuls start after
the first), and sign weights are unpacked on-device by DVE shift/and +
affine into resident 16-bit/fp8 operands (hidden under the first
gather). Output returns fp16; host casts to f32. Per-call axon traffic
~0.58GB vs ~2.2GB for naive full replication.

LN is folded around the matmul on raw x:
  out[t,o] = (x@bw.T)[t,o] * inv_t + a_t * S[o] + bias[o]
with S[o] = sum_d bw[o,d], inv_t = 1/(std_t+eps), a_t = -mu_t*inv_t;
stats/prep ride the host, inv/a arrive arranged [128, 64] (partition =
token%128, col = token chunk). a_t*S[o]+bias[o] is built once per
token chunk (cb); each PSUM eviction is psum*inv (+cb) straight to
fp16.

Modes (BITLIN_MODE):
  f16 — all-fp16 matmuls: stationary = gathered-x^T k-tile [128, 128],
        moving = weight slice [128, 512] (one fp32 PSUM bank), 32-deep
        k accumulation. 8192 MMs/core ~= the 16-bit PE roofline
        (~1.75 ms). rel err ~6e-4.
  dr  — (default) d < 2048 runs as single-fp8 DoubleRow pairs (2
        contraction rows/cycle, 8 MMs of K=256 at half cost), the rest
        fp16: MM stream ~0.63x of f16. rel err ~1.5e-2 (measured on
        the fixed seed-0 inputs) vs the 2e-2 gate.
"""

import os

import numpy as np
import ml_dtypes

import concourse.bass as bass
import concourse.tile as tile
from concourse import bacc
from concourse import mybir
from concourse.bass_utils import run_bass_kernel_spmd

F32 = mybir.dt.float32
F16 = mybir.dt.float16
FP8 = mybir.dt.float8e4
U8 = mybir.dt.uint8

NP_FP8 = ml_dtypes.float8_e4m3

T, D, O_FULL, NCORES = 8192, 4096, 16384, 8
O = O_FULL // NCORES  # 2048 out-features per core
TS = T // NCORES  # 1024 tokens shipped per core
OB = O // 8  # 256 packed-weight bytes per row
EPS = 1e-5

KT = D // 128  # 32 k-tiles
MC = T // 128  # 64 token chunks
NG = 8  # x^T AllGather chunks (pipelined with compute)
GT = TS // NG  # 128 local tokens per gather chunk == superblock == chunk
WN = 512  # moving free width (one PSUM bank at fp32)
NWCH = O // WN  # 4 out chunks

DF = 2048  # 'dr'/'drx': d < DF contracts as single-fp8 DoubleRow pairs
KQ = DF // 256  # 8 DoubleRow MMs (K=256 virtual each)
KH = (D - DF) // 128  # 16 upper k-tiles (fp16 in 'dr'; fp8 hi/lo in 'drx')

MODE = os.environ.get("BITLIN_MODE", "drx")


def _build_drx():
    """All-DoubleRow variant: d < DF as single-fp8 pairs (2 different d
    per cell), d >= DF as fp8 hi+lo pairs of the same d (near-fp16
    accuracy). One fp8 x^T tensor [DF + 2*(D-DF), TS] = [6144, TS]:
    row (2*kk+h)*128 + p <-> d = kk*256 + h*128 + p  (kk < 16, lower)
    row 4096 + (2*kk+h)*128 + p <-> (hi,lo)[h] of d = DF + kk*128 + p.
    Weight slots wall[:, s, :] (s = 0..47) mirror the same mapping,
    with upper sign rows duplicated across the pair dim. All 24 MMs
    per (chunk, oc) are K=256 DoubleRow at 0.5 cyc/row."""
    NR = 2 * D - DF  # 6144 shipped fp8 x rows
    NSL = NR // 128  # 48 slots
    nc = bacc.Bacc("TRN2", target_bir_lowering=False, debug=False)
    xq_d = nc.declare_dram_parameter("xq", [NR, TS], FP8, isOutput=False)
    w1_d = nc.declare_dram_parameter("w1", [D, OB], U8, isOutput=False)
    s_d = nc.declare_dram_parameter("srow", [O], F16, isOutput=False)
    b_d = nc.declare_dram_parameter("brow", [O], F16, isOutput=False)
    iv_d = nc.declare_dram_parameter("iv", [128, MC], F32, isOutput=False)
    aa_d = nc.declare_dram_parameter("aa", [128, MC], F32, isOutput=False)
    out_d = nc.declare_dram_parameter("out", [T, O], F16, isOutput=True)

    with tile.TileContext(nc) as tc:
        with (
            tc.tile_pool(name="dram", bufs=1, space="DRAM") as dram,
            tc.tile_pool(name="singles", bufs=1) as singles,
            tc.tile_pool(name="w1p", bufs=2) as w1p,
            tc.tile_pool(name="xtp", bufs=2) as xtp,
            tc.tile_pool(name="cbp", bufs=2) as cbp,
            tc.tile_pool(name="evp", bufs=4) as evp,
            tc.tile_pool(name="mmp", bufs=3, space="PSUM") as mmp,
        ):
            wall = singles.tile([128, NSL, O], FP8)  # all sign slots
            s_bc = singles.tile([128, O], F16)
            b_bc = singles.tile([128, O], F16)
            iv_sb = singles.tile([128, MC], F32)
            aa_sb = singles.tile([128, MC], F32)

            gxs = []
            for j in range(NG):
                ib = dram.tile([NR, GT], FP8, tag=f"ib{j}", name=f"ib{j}")
                gx = dram.tile(
                    [NCORES * NR, GT],
                    FP8,
                    tag=f"gx{j}",
                    name=f"gx{j}",
                    addr_space="Shared",
                )
                nc.gpsimd.dma_start(out=ib[:], in_=xq_d[:, j * GT : (j + 1) * GT])
                nc.gpsimd.collective_compute(
                    "AllGather",
                    mybir.AluOpType.bypass,
                    replica_groups=[list(range(NCORES))],
                    ins=[ib.opt()],
                    outs=[gx.opt()],
                )
                gxs.append(gx)

            nc.gpsimd.dma_start(out=iv_sb[:], in_=iv_d[:, :])
            nc.gpsimd.dma_start(out=aa_sb[:], in_=aa_d[:, :])
            sap = s_d[:]
            nc.gpsimd.dma_start(
                out=s_bc[:],
                in_=bass.AP(tensor=sap.tensor, offset=sap.offset, ap=[[0, 128]] + sap.ap),
            )
            bap = b_d[:]
            nc.gpsimd.dma_start(
                out=b_bc[:],
                in_=bass.AP(tensor=bap.tensor, offset=bap.offset, ap=[[0, 128]] + bap.ap),
            )

            # unpack packed sign bits into wall slots
            for r in range(KT):  # w1 row-tile r covers d = r*128..+127
                w1 = w1p.tile([128, OB], U8, tag="w1")
                nc.gpsimd.dma_start(
                    out=w1[:], in_=w1_d[r * 128 : (r + 1) * 128, :]
                )
                slots = [r] if r < DF // 128 else [2 * r - 16, 2 * r - 15]
                for b in range(8):
                    tb = w1p.tile([128, OB], U8, tag="tb")
                    nc.vector.tensor_scalar(
                        tb[:],
                        w1[:],
                        b,
                        1,
                        op0=mybir.AluOpType.logical_shift_right,
                        op1=mybir.AluOpType.bitwise_and,
                    )
                    for s in slots:
                        nc.vector.tensor_scalar(
                            wall[:, s, b * OB : (b + 1) * OB],
                            tb[:],
                            -2.0,
                            1.0,
                            op0=mybir.AluOpType.mult,
                            op1=mybir.AluOpType.add,
                        )

            for j in range(NG):
                for cg in range(NCORES):
                    m = cg * NG + j
                    xqb = xtp.tile([128, NSL, GT], FP8, tag="xqb")
                    gap = gxs[j][:]
                    nc.gpsimd.dma_start(
                        out=xqb[:],
                        in_=bass.AP(
                            tensor=gap.tensor,
                            offset=gap.offset + cg * NR * GT,
                            ap=[[GT, 128], [128 * GT, NSL], [1, GT]],
                        ),
                    )
                    cb = cbp.tile([128, O], F16, tag="cb")
                    nc.vector.tensor_scalar_mul(cb[:], s_bc[:], aa_sb[:, m : m + 1])
                    nc.vector.tensor_add(cb[:], cb[:], b_bc[:])
                    for oc in range(NWCH):
                        osl = slice(oc * WN, (oc + 1) * WN)
                        pm = mmp.tile([128, WN], F32, tag="mm")
                        for kk in range(NSL // 2):  # 24 DoubleRow MMs
                            nc.tensor.matmul(
                                pm[:],
                                xqb[:, 2 * kk : 2 * kk + 2, :],
                                wall[:, 2 * kk : 2 * kk + 2, osl],
                                start=(kk == 0),
                                stop=(kk == NSL // 2 - 1),
                                perf_mode=mybir.MatmulPerfMode.DoubleRow,
                                skip_group_check=True,
                            )
                        ev = evp.tile([128, WN], F16, tag="ev")
                        nc.vector.tensor_scalar_mul(ev[:], pm[:], iv_sb[:, m : m + 1])
                        nc.vector.tensor_add(ev[:], ev[:], cb[:, osl])
                        nc.gpsimd.dma_start(
                            out=out_d[m * 128 : (m + 1) * 128, osl], in_=ev[:]
                        )
    nc.compile()
    return nc


def _build(mode):
    if mode == "drx":
        return _build_drx()
    dr = mode == "dr"
    nc = bacc.Bacc("TRN2", target_bir_lowering=False, debug=False)
    # fp16 x^T rows: all of d in f16 mode; d >= DF in dr mode
    DH = D - DF if dr else D
    xt_d = nc.declare_dram_parameter("xt", [DH, TS], F16, isOutput=False)
    if dr:
        xq_d = nc.declare_dram_parameter("xq", [DF, TS], FP8, isOutput=False)
    w1_d = nc.declare_dram_parameter("w1", [D, OB], U8, isOutput=False)
    s_d = nc.declare_dram_parameter("srow", [O], F16, isOutput=False)
    b_d = nc.declare_dram_parameter("brow", [O], F16, isOutput=False)
    iv_d = nc.declare_dram_parameter("iv", [128, MC], F32, isOutput=False)
    aa_d = nc.declare_dram_parameter("aa", [128, MC], F32, isOutput=False)
    out_d = nc.declare_dram_parameter("out", [T, O], F16, isOutput=True)

    with tile.TileContext(nc) as tc:
        with (
            tc.tile_pool(name="dram", bufs=1, space="DRAM") as dram,
            tc.tile_pool(name="singles", bufs=1) as singles,
            tc.tile_pool(name="w1p", bufs=2) as w1p,
            tc.tile_pool(name="xtp", bufs=2) as xtp,
            tc.tile_pool(name="cbp", bufs=2) as cbp,
            tc.tile_pool(name="evp", bufs=4) as evp,
            tc.tile_pool(name="mmp", bufs=3, space="PSUM") as mmp,
        ):
            # resident sign(w)^T: fp16 half (+ fp8 DoubleRow half in dr mode)
            KTH = KH if dr else KT
            wt = singles.tile([128, KTH, O], F16)
            if dr:
                # dim1 = kk*2 + h; row r = kk*256 + h*128 + p <-> d = r
                wq = singles.tile([128, 2 * KQ, O], FP8)
            s_bc = singles.tile([128, O], F16)  # S broadcast across partitions
            b_bc = singles.tile([128, O], F16)  # bias broadcast
            iv_sb = singles.tile([128, MC], F32)
            aa_sb = singles.tile([128, MC], F32)

            # ---- x^T shard -> bounce -> chunked AllGather over the 8 cores
            gxts, gxqs = [], []
            for j in range(NG):
                if dr:
                    ibq = dram.tile([DF, GT], FP8, tag=f"ibq{j}", name=f"ibq{j}")
                    gxq = dram.tile(
                        [NCORES * DF, GT],
                        FP8,
                        tag=f"gxq{j}",
                        name=f"gxq{j}",
                        addr_space="Shared",
                    )
                    nc.gpsimd.dma_start(
                        out=ibq[:], in_=xq_d[:, j * GT : (j + 1) * GT]
                    )
                    nc.gpsimd.collective_compute(
                        "AllGather",
                        mybir.AluOpType.bypass,
                        replica_groups=[list(range(NCORES))],
                        ins=[ibq.opt()],
                        outs=[gxq.opt()],
                    )
                    gxqs.append(gxq)
                ibt = dram.tile([DH, GT], F16, tag=f"ibt{j}", name=f"ibt{j}")
                gxt = dram.tile(
                    [NCORES * DH, GT],
                    F16,
                    tag=f"gxt{j}",
                    name=f"gxt{j}",
                    addr_space="Shared",
                )
                nc.gpsimd.dma_start(out=ibt[:], in_=xt_d[:, j * GT : (j + 1) * GT])
                nc.gpsimd.collective_compute(
                    "AllGather",
                    mybir.AluOpType.bypass,
                    replica_groups=[list(range(NCORES))],
                    ins=[ibt.opt()],
                    outs=[gxt.opt()],
                )
                gxts.append(gxt)

            # ---- prep: tiny stat/bias loads + partition-broadcast rows ----
            nc.gpsimd.dma_start(out=iv_sb[:], in_=iv_d[:, :])
            nc.gpsimd.dma_start(out=aa_sb[:], in_=aa_d[:, :])
            sap = s_d[:]
            nc.gpsimd.dma_start(
                out=s_bc[:],
                in_=bass.AP(tensor=sap.tensor, offset=sap.offset, ap=[[0, 128]] + sap.ap),
            )
            bap = b_d[:]
            nc.gpsimd.dma_start(
                out=b_bc[:],
                in_=bass.AP(tensor=bap.tensor, offset=bap.offset, ap=[[0, 128]] + bap.ap),
            )

            # ---- prep: land packed sign bits, unpack to resident operands
            # byte j bit b of row d <-> sign(w)[o = b*256 + j, d] < 0
            def unpack_rows(dst, r0, nrt, to_dtype):
                # dst[:, r, b*OB:(b+1)*OB] <- +-1 of w1 rows [(r0+r)*128, +128)
                for r in range(nrt):
                    w1 = w1p.tile([128, OB], U8, tag="w1")
                    nc.gpsimd.dma_start(
                        out=w1[:],
                        in_=w1_d[(r0 + r) * 128 : (r0 + r + 1) * 128, :],
                    )
                    for b in range(8):
                        tb = w1p.tile([128, OB], U8, tag="tb")
                        nc.vector.tensor_scalar(
                            tb[:],
                            w1[:],
                            b,
                            1,
                            op0=mybir.AluOpType.logical_shift_right,
                            op1=mybir.AluOpType.bitwise_and,
                        )
                        nc.vector.tensor_scalar(
                            dst[:, r, b * OB : (b + 1) * OB],
                            tb[:],
                            -2.0,
                            1.0,
                            op0=mybir.AluOpType.mult,
                            op1=mybir.AluOpType.add,
                        )

            if dr:
                unpack_rows(wq, 0, 2 * KQ, FP8)  # rows d < DF -> fp8 pairs
                unpack_rows(wt, DF // 128, KH, F16)  # rows d >= DF -> fp16
            else:
                unpack_rows(wt, 0, KT, F16)

            # ---- main: stream gathered x^T superblocks, matmul, fused evict
            for j in range(NG):
                for cg in range(NCORES):
                    m = cg * NG + j  # token chunk (tokens cg*TS + j*GT ..)
                    xtb = xtp.tile([128, KTH, GT], F16, tag="xtb")
                    gap = gxts[j][:]
                    nc.gpsimd.dma_start(
                        out=xtb[:],
                        in_=bass.AP(
                            tensor=gap.tensor,
                            offset=gap.offset + cg * DH * GT,
                            ap=[[GT, 128], [128 * GT, KTH], [1, GT]],
                        ),
                    )
                    if dr:
                        xqb = xtp.tile([128, 2 * KQ, GT], FP8, tag="xqb")
                        qap = gxqs[j][:]
                        nc.gpsimd.dma_start(
                            out=xqb[:],
                            in_=bass.AP(
                                tensor=qap.tensor,
                                offset=qap.offset + cg * DF * GT,
                                ap=[[GT, 128], [128 * GT, 2 * KQ], [1, GT]],
                            ),
                        )
                    # cb[p, o] = a_t * S[o] + bias[o] for this token chunk
                    cb = cbp.tile([128, O], F16, tag="cb")
                    nc.vector.tensor_scalar_mul(cb[:], s_bc[:], aa_sb[:, m : m + 1])
                    nc.vector.tensor_add(cb[:], cb[:], b_bc[:])
                    for oc in range(NWCH):
                        osl = slice(oc * WN, (oc + 1) * WN)
                        pm = mmp.tile([128, WN], F32, tag="mm")
                        if dr:
                            for kk in range(KQ):
                                nc.tensor.matmul(
                                    pm[:],
                                    xqb[:, 2 * kk : 2 * kk + 2, :],
                                    wq[:, 2 * kk : 2 * kk + 2, osl],
                                    start=(kk == 0),
                                    stop=False,
                                    perf_mode=mybir.MatmulPerfMode.DoubleRow,
                                    skip_group_check=True,
                                )
                        for k in range(KTH):
                            nc.tensor.matmul(
                                pm[:],
                                xtb[:, k, :],
                                wt[:, k, osl],
                                start=(k == 0 and not dr),
                                stop=(k == KTH - 1),
                                skip_group_check=True,
                            )
                        ev = evp.tile([128, WN], F16, tag="ev")
                        nc.vector.tensor_scalar_mul(ev[:], pm[:], iv_sb[:, m : m + 1])
                        nc.vector.tensor_add(ev[:], ev[:], cb[:, osl])
                        nc.gpsimd.dma_start(
                            out=out_d[m * 128 : (m + 1) * 128, osl], in_=ev[:]
                        )
    nc.compile()
    return nc


_NC_CACHE = {}
LAST_RESULTS = None


def kernel(x, weight, bias):
    global LAST_RESULTS
    import time as _time

    _tv = os.environ.get("BITLIN_TIME", "0") == "1"
    _t0 = _time.time()
    x = np.asarray(x, dtype=np.float32)
    weight = np.asarray(weight, dtype=np.float32)
    bias = np.asarray(bias, dtype=np.float32)
    dr = MODE == "dr"

    # LN stats folded to a per-token affine: out = (x@bw.T)*inv + a*S + b
    mu = x.mean(axis=1, dtype=np.float64)
    sd = np.sqrt(x.var(axis=1, ddof=1, dtype=np.float64))
    inv = (1.0 / (sd + EPS)).astype(np.float32)
    aa = (-mu * inv).astype(np.float32)
    iv_t = np.ascontiguousarray(inv.reshape(MC, 128).T)  # [128, chunk]
    aa_t = np.ascontiguousarray(aa.reshape(MC, 128).T)

    if MODE == "drx":
        # one fp8 tensor: lower DF rows single-fp8; upper as hi/lo pairs
        # interleaved per 128-row tile (row (2kk+h)*128+p, h = hi|lo)
        xQ = np.empty((2 * D - DF, T), NP_FP8)
        xQ[:DF] = x[:, :DF].T.astype(NP_FP8, order="C")
        xu = x[:, DF:]
        hi = xu.T.astype(NP_FP8, order="C")  # [D-DF, T]
        lo = (np.ascontiguousarray(xu.T) - hi.astype(np.float32)).astype(NP_FP8)
        up = xQ[DF:].reshape(KH, 2, 128, T)
        up[:, 0] = hi.reshape(KH, 128, T)
        up[:, 1] = lo.reshape(KH, 128, T)
    elif dr:
        xT = x[:, DF:].T.astype(np.float16, order="C")  # [D-DF, T]
        xQ = x[:, :DF].T.astype(NP_FP8, order="C")  # [DF, T] single fp8
    else:
        xT = x.T.astype(np.float16, order="C")  # [D, T] fp16

    neg = weight < 0  # sign bit; reference asserts no exact zeros
    S = (D - 2 * neg.sum(axis=1, dtype=np.int32)).astype(np.float16)  # [O_FULL]
    b16 = bias.astype(np.float16)
    # pack: byte j bit b of (core i, row d) <-> neg[i*2048 + b*256 + j, d]
    pk = np.packbits(neg.reshape(NCORES, 8, OB, D), axis=1, bitorder="little")
    pk = pk[:, 0]  # [NCORES, OB, D]

    if _tv:
        print(f"[bitlin] host prep: {_time.time() - _t0:.2f}s", flush=True)
    if MODE not in _NC_CACHE:
        _NC_CACHE[MODE] = _build(MODE)
    nc = _NC_CACHE[MODE]
    _t1 = _time.time()

    in_maps = []
    for i in range(NCORES):
        m = {
            "w1": pk[i].T,
            "srow": S[i * O : (i + 1) * O],
            "brow": b16[i * O : (i + 1) * O],
            "iv": iv_t,
            "aa": aa_t,
        }
        if MODE == "drx":
            m["xq"] = xQ[:, i * TS : (i + 1) * TS]
        else:
            m["xt"] = xT[:, i * TS : (i + 1) * TS]
            if dr:
                m["xq"] = xQ[:, i * TS : (i + 1) * TS]
        in_maps.append(m)
    res = run_bass_kernel_spmd(nc, in_maps, list(range(NCORES)))
    LAST_RESULTS = res
    if _tv:
        print(f"[bitlin] run_bass: {_time.time() - _t1:.2f}s", flush=True)
    out = np.empty((T, O_FULL), np.float32)
    for i in range(NCORES):
        out[:, i * O : (i + 1) * O] = res.results[i]["out"]
    if _tv:
        print(f"[bitlin] out assemble: {_time.time() - _t1:.2f}s cum", flush=True)
    return out
